# revision 23
# baseline (speedup 1.0000x reference)
"""Trainium2 Bass kernel for dense_cnn problem (v2: split depthwise off PE).

Math (per batch element n, C=128 channels, H=W=56, G=8):
  t1 = conv_h(x, w1)          5-tap conv over H with full channel mixing
  t3 = dwconv_h(t1, w3)       3-tap depthwise conv over H
  t4[g] = sum_{c,k} x[c, h, w+2k-2] * w4[c,k,g]   (3 width taps, dil 2)
  out[c] = t3[c] * t4[c % 8]

Device strategy (data-parallel, 4 batch elems per core across 8 cores):
  - PE matmul time is output-columns only (1 col/cycle at 2.4 GHz), so
    the folded 7-tap t3 conv of the first version wasted 2 full passes
    over x.  Here PE computes only t1 (5 taps) + t4 (3 taps): 98k
    columns/core ~= 41 us, and the trace shows it 100% packed
    (LDWEIGHTS hides under the 448-col streams).
  - Act drains PSUM->SBUF as fp16, one instr per 16-row super-chunk
    spanning 2 PSUM banks (gpsimd cannot touch PSUM, and its ALU ops
    are ~15 ns/elem/lane - useless for real work).
  - The 3-tap depthwise + final multiply run on DVE in fp16:
    three tensor_scalar scaled copies (4x perf mode, 0.26 ns/elem/lane)
    + two tensor_tensor adds and the t3*t4 multiply (2x mode).  The
    fused scalar_tensor_tensor is 1x-only on HW - slower than ts+tt.
    t1s carries a zero row on each side so borders need no special
    casing.  Work is sliced in 28-row halves; the last batch elem uses
    per-super slices with its center tap pushed to Act so only a short
    chain trails the final matmul.
  - Everything on-chip is fp16 (values stay < 100; rel err ~1e-3), PSUM
    accumulates fp32; output leaves the device fp16, host upcasts.
  - Dummy warm-up matmuls trip the PE_HAM clock gate (1.2 -> 2.4 GHz)
    while the first DMAs stream in; w1 and two small x pieces (10+8
    rows) lead the DMA queue so real matmuls start as early as possible
    and never leave a PE gap (a gap also resets the clock-gate window).
"""

import sys

sys.path.insert(0, "/opt/trn_rl_repo")

import ml_dtypes
import numpy as np

import concourse.bacc as bacc
import concourse.bass as bass
import concourse.mybir as mybir
import concourse.tile as tile
from concourse import bass_utils

N, C, H, W, G = 32, 128, 56, 56, 8
NCORES = 8
NPC = N // NCORES   # batch elems per core
SCH = 16            # rows per super-chunk (PSUM tile = 2 banks)
SUPERS = [(0, 16), (16, 32), (32, 48), (48, 56)]
HALVES = [(0, 28), (28, 56)]

F32 = mybir.dt.float32
F16 = mybir.dt.float16

WARMUP_N = 14
WARMUP_COLS = 256

TRACE = False
TRACE_DIR = None
LAST_EXEC_NS = None
LAST_RESULTS = None

_COMPILED = None


def _enable_trace_hook():
    """The agent image's ``antenv`` lacks ``axon_hooks``, so the boot-time
    NTFF hook registration silently degraded. Recreate the module and
    register the same ctypes-based hook; also skip the bucket upload."""
    import sys as _sys
    import types

    if "antenv.axon_hooks" not in _sys.modules:
        mod = types.ModuleType("antenv.axon_hooks")
        mod._hook = None

        def set_axon_ntff_profile_hook(h):
            mod._hook = h

        def get_axon_ntff_profile_hook():
            return mod._hook

        mod.set_axon_ntff_profile_hook = set_axon_ntff_profile_hook
        mod.get_axon_ntff_profile_hook = get_axon_ntff_profile_hook
        _sys.modules["antenv.axon_hooks"] = mod
        import antenv

        antenv.axon_hooks = mod

    from antenv.axon_hooks import get_axon_ntff_profile_hook as _get

    if _get() is None:
        from trn_agent_boot.trn_boot import _ntff_profile_via_ctypes

        hook = _ntff_profile_via_ctypes("/opt/axon/libaxon_pjrt.so")
        if hook is not None:
            _sys.modules["antenv.axon_hooks"].set_axon_ntff_profile_hook(hook)

    bass_utils.upload_artifacts = lambda tmpdir: f"local:{tmpdir}"


def _build():
    nc = bacc.Bacc(
        "TRN2",
        target_bir_lowering=False,
        debug=False,
        enable_asserts=False,
        num_devices=NCORES,
    )

    x_d = nc.dram_tensor("x_s", (NPC, C, H, W), F16, kind="ExternalInput").ap()
    w1_d = nc.dram_tensor("w1t", (C, 5, C), F16, kind="ExternalInput").ap()
    w4_d = nc.dram_tensor("w4b", (C, 3, C), F16, kind="ExternalInput").ap()
    w3_d = nc.dram_tensor("w3v", (C, 3), F32, kind="ExternalInput").ap()
    out_d = nc.dram_tensor("out", (NPC, C, H, W), F16, kind="ExternalOutput").ap()

    MUL = mybir.AluOpType.mult
    ADD = mybir.AluOpType.add
    COPY = mybir.ActivationFunctionType.Copy

    with tile.TileContext(nc) as tc:
        with (
            tc.tile_pool(name="wpool", bufs=1) as wpool,
            tc.tile_pool(name="xpool", bufs=1) as xpool,
            tc.tile_pool(name="spool", bufs=1) as spool,
            tc.tile_pool(name="tpool", bufs=2) as tpool,
            tc.tile_pool(name="psT", bufs=2, space="PSUM") as ptpool,
            tc.tile_pool(name="psB", bufs=2, space="PSUM") as pbpool,
        ):
            # Warm-up matmuls on an SBUF scratch region zeroed by gpsimd
            # (gpsimd starts earliest after the NEFF preamble). PE_HAM
            # ungates the 2.4 GHz clock only after ~3us of sustained
            # activity; results land in PSUM banks that are overwritten
            # by the first real accumulations.
            dmy = wpool.tile([C, WARMUP_COLS], F16)
            nc.gpsimd.memset(dmy[:], 0.0)
            wps = []
            for i in range(WARMUP_N):
                wp = pbpool.tile([C, 2, 512], F32, name="pb")
                nc.tensor.matmul(
                    wp[:, 0, 0:WARMUP_COLS],
                    lhsT=dmy[:, 0:C],
                    rhs=dmy[:],
                    start=True,
                    stop=True,
                )
                wps.append(wp)

            # full-batch SBUF tensors (one tile each, 128 partitions first)
            xall = xpool.tile([C, NPC, H, W], F16)
            t1s = spool.tile([C, NPC, H + 2, W], F16)   # zero row top+bottom
            t4s = spool.tile([C, NPC, H, W], F16)
            osb = spool.tile([C, NPC, H, W], F16)

            # zero the t1s padding rows (rows 0 and 57 of each batch elem)
            nc.gpsimd.memset(t1s[:, :, 0, :], 0.0)
            nc.gpsimd.memset(t1s[:, :, H + 1, :], 0.0)

            w1t = wpool.tile([C, 5, C], F16)
            w4t = wpool.tile([C, 3, C], F16)
            w3t = wpool.tile([C, 3], F32)

            # input DMAs: w1 (tiny) and two small x pieces lead so the
            # first chunks unblock as early as possible even when the DMA
            # path has a slow day (a PE gap costs ~2.7x its length: the
            # idle itself plus the HAM clock-gate window it resets).
            # chunk 0 needs x rows <=10, chunk 1 rows <=18.
            nc.sync.dma_start(w1t[:], w1_d[:])
            nc.sync.dma_start(xall[:, 0, 0:10, :], x_d[0, :, 0:10, :])
            nc.sync.dma_start(xall[:, 0, 10:18, :], x_d[0, :, 10:18, :])
            # remainder in two pieces: chunk 2+ would otherwise wait for
            # the whole 1.1MB tail's completion semaphore (~13.5us) and
            # stall the PE mid-stream (seen as 0.2-0.5us hiccups)
            nc.sync.dma_start(xall[:, 0, 18:36, :], x_d[0, :, 18:36, :])
            nc.sync.dma_start(xall[:, 0, 36:H, :], x_d[0, :, 36:H, :])
            nc.sync.dma_start(w4t[:], w4_d[:])
            nc.sync.dma_start(w3t[:], w3_d[:])
            for n in range(1, NPC):
                nc.sync.dma_start(xall[:, n, :, :], x_d[n])

            SUPERS_LAST = [(0, 16), (16, 32), (32, 40), (40, 48), (48, 56)]
            SLICES_LAST = [(0, 15), (15, 31), (31, 39), (39, 47), (47, 56)]

            for n in range(NPC):
                xn = xall[:, n]
                last = n == NPC - 1

                for si, (h0, h1) in enumerate(SUPERS_LAST if last else SUPERS):
                    pt = ptpool.tile([C, 2, 512], F32, name="pt")
                    pb = pbpool.tile([C, 2, 512], F32, name="pb")
                    nq = (h1 - h0) // 8
                    for q in range(nq):
                        c0 = h0 + q * 8
                        # ---- t1: 5-tap conv over H (zero pad via row clip)
                        # center tap e=2 first: always full 8 rows
                        mmsT = []
                        for e in (2, 0, 1, 3, 4):
                            o_lo = max(0, 2 - e - c0)
                            o_hi = min(8, H + 2 - e - c0)
                            mmsT.append(
                                (
                                    w1t[:, e, :],
                                    xn[:, c0 + o_lo + e - 2 : c0 + o_hi + e - 2, :],
                                    pt[:, q, o_lo * W : o_hi * W],
                                )
                            )
                        for i, (lhsT, rhs, outap) in enumerate(mmsT):
                            nc.tensor.matmul(
                                outap,
                                lhsT=lhsT,
                                rhs=rhs,
                                start=(i == 0),
                                stop=(i == len(mmsT) - 1),
                            )
                        # ---- t4: 3 width taps (dilation 2), col-clipped
                        pbq = pb[:, q, 0 : 8 * W].rearrange(
                            "p (h w) -> p h w", h=8, w=W
                        )
                        rows = xn[:, c0 : c0 + 8, :]
                        mmsB = [
                            (w4t[:, 1, :], rows, pbq[:, :, :]),
                            (w4t[:, 0, :], xn[:, c0 : c0 + 8, 0 : W - 2], pbq[:, :, 2:W]),
                            (w4t[:, 2, :], xn[:, c0 : c0 + 8, 2:W], pbq[:, :, 0 : W - 2]),
                        ]
                        for i, (lhsT, rhs, outap) in enumerate(mmsB):
                            nc.tensor.matmul(
                                outap,
                                lhsT=lhsT,
                                rhs=rhs,
                                start=(i == 0),
                                stop=(i == len(mmsB) - 1),
                            )

                    # PSUM -> SBUF fp16 on Act (gpsimd cannot touch PSUM);
                    # one instr per super-chunk spans both banks
                    nc.scalar.copy(
                        t1s[:, n, h0 + 1 : h1 + 1, :],
                        pt[:, 0:nq, 0 : 8 * W],
                    )
                    if not last:
                        nc.scalar.copy(
                            t4s[:, n, h0:h1, :],
                            pb[:, 0:nq, 0 : 8 * W],
                        )

                    # Depthwise+combine chain, all on DVE: three scaled
                    # copies in the 4x tensor_scalar mode, adds and final
                    # multiply in the 2x tensor_tensor mode (the fused
                    # scalar_tensor_tensor is 1x-only: slower).  Row slices
                    # are sized so each becomes ready as its supers land;
                    # the last batch elem uses 14-row quarters so only one
                    # small chain trails the final matmul.
                    def emit_t4s_copy():
                        nc.scalar.copy(
                            t4s[:, n, h0:h1, :],
                            pb[:, 0:nq, 0 : 8 * W],
                        )

                    def emit_dw(r0, r1, tb_on_act=False, mid_copy=None, split_out=False):
                        nr = r1 - r0
                        ta = tpool.tile([C, nr, W], F16, name="ta")
                        tb = tpool.tile([C, nr, W], F16, name="tb")
                        tc_ = tpool.tile([C, nr, W], F16, name="tc_")
                        ts_ = tpool.tile([C, nr, W], F16, name="ts_")
                        t3 = tpool.tile([C, nr, W], F16, name="t3")
                        nc.vector.tensor_scalar_mul(
                            ta[:], t1s[:, n, r0 : r1, :], w3t[:, 0:1]
                        )
                        if tb_on_act:
                            nc.scalar.activation(
                                tb[:],
                                t1s[:, n, r0 + 1 : r1 + 1, :],
                                COPY,
                                scale=w3t[:, 1:2],
                            )
                        else:
                            nc.vector.tensor_scalar_mul(
                                tb[:], t1s[:, n, r0 + 1 : r1 + 1, :], w3t[:, 1:2]
                            )
                        if mid_copy is not None:
                            mid_copy()
                        nc.vector.tensor_scalar_mul(
                            tc_[:], t1s[:, n, r0 + 2 : r1 + 2, :], w3t[:, 2:3]
                        )
                        nc.vector.tensor_tensor(ts_[:], ta[:], tb[:], op=ADD)
                        nc.vector.tensor_tensor(t3[:], ts_[:], tc_[:], op=ADD)
                        cuts = [r0, (r0 + r1) // 2, r1] if split_out else [r0, r1]
                        for a, b in zip(cuts, cuts[1:]):
                            nc.vector.tensor_tensor(
                                osb[:, n, a:b, :],
                                t3[:, a - r0 : b - r0, :],
                                t4s[:, n, a:b, :],
                                op=MUL,
                            )
                            nc.sync.dma_start(
                                out_d[n, :, a:b, :], osb[:, n, a:b, :]
                            )

                    if not last:
                        if si in (1, 3):
                            # elem 2's tail half helps drain the DVE backlog
                            # by pushing its center tap to Act
                            emit_dw(*HALVES[si // 2],
                                    tb_on_act=(n == NPC - 2 and si == 3))
                    else:
                        # last elem: slices aligned to supers so each chain
                        # launches as soon as its own super is copied; the
                        # t4 copy rides the Act queue between tb and the
                        # adds, and only a 9-row chain trails the final
                        # matmul
                        emit_dw(*SLICES_LAST[si], tb_on_act=True,
                                mid_copy=emit_t4s_copy)

    nc.compile()
    return nc


def _get_compiled():
    global _COMPILED
    if _COMPILED is None:
        _COMPILED = _build()
    return _COMPILED


def _prep_weights(w1, w3, w4):
    f16 = ml_dtypes.float16 if hasattr(ml_dtypes, "float16") else np.float16
    w1c = np.asarray(w1, dtype=np.float32)[:, :, :, 0]          # (co, ci, e)
    w1t = np.ascontiguousarray(np.transpose(w1c, (1, 2, 0)))    # (ci, e, co)
    w3c = np.asarray(w3, dtype=np.float32)[:, 0, :, 0]          # (c, d)
    w4c = np.asarray(w4, dtype=np.float32)[:, :, 0, :]          # (ci, k, g)
    w4b = np.ascontiguousarray(np.tile(w4c, (1, 1, C // G)))    # (ci, k, 128)
    return (
        w1t.astype(np.float16),
        w4b.astype(np.float16),
        np.ascontiguousarray(w3c).astype(np.float32),
    )


def kernel(x, w1, w3, w4):
    global LAST_EXEC_NS, LAST_RESULTS
    nc = _get_compiled()
    xh = np.ascontiguousarray(np.asarray(x, dtype=np.float32)).astype(np.float16)
    w1t, w4b, w3v = _prep_weights(w1, w3, w4)

    in_maps = [
        {
            "x_s": np.ascontiguousarray(xh[i * NPC : (i + 1) * NPC]),
            "w1t": w1t,
            "w4b": w4b,
            "w3v": w3v,
        }
        for i in range(NCORES)
    ]
    if TRACE:
        _enable_trace_hook()
    res = bass_utils.run_bass_kernel_spmd(
        nc,
        in_maps,
        core_ids=list(range(NCORES)),
        trace=TRACE,
        tmpdir=TRACE_DIR,
    )
    LAST_EXEC_NS = res.exec_time_ns
    LAST_RESULTS = res
    out = np.concatenate([res.results[i]["out"] for i in range(NCORES)], axis=0)
    return out.astype(np.float32)


# revision 24
# speedup vs baseline: 1.0388x; 1.0388x over previous
"""Trainium2 Bass kernel for dense_cnn problem (v2: split depthwise off PE).

Math (per batch element n, C=128 channels, H=W=56, G=8):
  t1 = conv_h(x, w1)          5-tap conv over H with full channel mixing
  t3 = dwconv_h(t1, w3)       3-tap depthwise conv over H
  t4[g] = sum_{c,k} x[c, h, w+2k-2] * w4[c,k,g]   (3 width taps, dil 2)
  out[c] = t3[c] * t4[c % 8]

Device strategy (data-parallel, 4 batch elems per core across 8 cores):
  - PE matmul time is output-columns only (1 col/cycle at 2.4 GHz), so
    the folded 7-tap t3 conv of the first version wasted 2 full passes
    over x.  Here PE computes only t1 (5 taps) + t4 (3 taps): 98k
    columns/core ~= 41 us, and the trace shows it 100% packed
    (LDWEIGHTS hides under the 448-col streams).
  - Act drains PSUM->SBUF as fp16, one instr per 16-row super-chunk
    spanning 2 PSUM banks (gpsimd cannot touch PSUM, and its ALU ops
    are ~15 ns/elem/lane - useless for real work).
  - The 3-tap depthwise + final multiply run on DVE in fp16:
    three tensor_scalar scaled copies (4x perf mode, 0.26 ns/elem/lane)
    + two tensor_tensor adds and the t3*t4 multiply (2x mode).  The
    fused scalar_tensor_tensor is 1x-only on HW - slower than ts+tt.
    t1s carries a zero row on each side so borders need no special
    casing.  Work is sliced in 28-row halves; the last batch elem uses
    per-super slices with its center tap pushed to Act so only a short
    chain trails the final matmul.
  - Everything on-chip is fp16 (values stay < 100; rel err ~1e-3), PSUM
    accumulates fp32; output leaves the device fp16, host upcasts.
  - Dummy warm-up matmuls trip the PE_HAM clock gate (1.2 -> 2.4 GHz)
    while the first DMAs stream in; w1 and two small x pieces (10+8
    rows) lead the DMA queue so real matmuls start as early as possible
    and never leave a PE gap (a gap also resets the clock-gate window).
"""

import sys

sys.path.insert(0, "/opt/trn_rl_repo")

import ml_dtypes
import numpy as np

import concourse.bacc as bacc
import concourse.bass as bass
import concourse.mybir as mybir
import concourse.tile as tile
from concourse import bass_utils

N, C, H, W, G = 32, 128, 56, 56, 8
NCORES = 8
NPC = N // NCORES   # batch elems per core
SCH = 16            # rows per super-chunk (PSUM tile = 2 banks)
SUPERS = [(0, 16), (16, 32), (32, 48), (48, 56)]
HALVES = [(0, 28), (28, 56)]

F32 = mybir.dt.float32
F16 = mybir.dt.float16

WARMUP_N = 20
WARMUP_COLS = 256

TRACE = False
TRACE_DIR = None
LAST_EXEC_NS = None
LAST_RESULTS = None

_COMPILED = None


def _enable_trace_hook():
    """The agent image's ``antenv`` lacks ``axon_hooks``, so the boot-time
    NTFF hook registration silently degraded. Recreate the module and
    register the same ctypes-based hook; also skip the bucket upload."""
    import sys as _sys
    import types

    if "antenv.axon_hooks" not in _sys.modules:
        mod = types.ModuleType("antenv.axon_hooks")
        mod._hook = None

        def set_axon_ntff_profile_hook(h):
            mod._hook = h

        def get_axon_ntff_profile_hook():
            return mod._hook

        mod.set_axon_ntff_profile_hook = set_axon_ntff_profile_hook
        mod.get_axon_ntff_profile_hook = get_axon_ntff_profile_hook
        _sys.modules["antenv.axon_hooks"] = mod
        import antenv

        antenv.axon_hooks = mod

    from antenv.axon_hooks import get_axon_ntff_profile_hook as _get

    if _get() is None:
        from trn_agent_boot.trn_boot import _ntff_profile_via_ctypes

        hook = _ntff_profile_via_ctypes("/opt/axon/libaxon_pjrt.so")
        if hook is not None:
            _sys.modules["antenv.axon_hooks"].set_axon_ntff_profile_hook(hook)

    bass_utils.upload_artifacts = lambda tmpdir: f"local:{tmpdir}"


def _build():
    nc = bacc.Bacc(
        "TRN2",
        target_bir_lowering=False,
        debug=False,
        enable_asserts=False,
        num_devices=NCORES,
    )

    x_d = nc.dram_tensor("x_s", (NPC, C, H, W), F16, kind="ExternalInput").ap()
    w1_d = nc.dram_tensor("w1t", (C, 5, C), F16, kind="ExternalInput").ap()
    w4_d = nc.dram_tensor("w4b", (C, 3, C), F16, kind="ExternalInput").ap()
    w3_d = nc.dram_tensor("w3v", (C, 3), F32, kind="ExternalInput").ap()
    out_d = nc.dram_tensor("out", (NPC, C, H, W), F16, kind="ExternalOutput").ap()

    MUL = mybir.AluOpType.mult
    ADD = mybir.AluOpType.add
    COPY = mybir.ActivationFunctionType.Copy

    with tile.TileContext(nc) as tc:
        with (
            tc.tile_pool(name="wpool", bufs=1) as wpool,
            tc.tile_pool(name="xpool", bufs=1) as xpool,
            tc.tile_pool(name="spool", bufs=1) as spool,
            tc.tile_pool(name="tpool", bufs=2) as tpool,
            tc.tile_pool(name="psT", bufs=2, space="PSUM") as ptpool,
            tc.tile_pool(name="psB", bufs=2, space="PSUM") as pbpool,
        ):
            # Warm-up matmuls on an SBUF scratch region zeroed by gpsimd
            # (gpsimd starts earliest after the NEFF preamble). PE_HAM
            # ungates the 2.4 GHz clock only after ~3us of sustained
            # activity; results land in PSUM banks that are overwritten
            # by the first real accumulations.
            dmy = wpool.tile([C, WARMUP_COLS], F16)
            nc.gpsimd.memset(dmy[:], 0.0)
            wps = []
            for i in range(WARMUP_N):
                wp = pbpool.tile([C, 2, 512], F32, name="pb")
                nc.tensor.matmul(
                    wp[:, 0, 0:WARMUP_COLS],
                    lhsT=dmy[:, 0:C],
                    rhs=dmy[:],
                    start=True,
                    stop=True,
                )
                wps.append(wp)

            # full-batch SBUF tensors (one tile each, 128 partitions first)
            xall = xpool.tile([C, NPC, H, W], F16)
            t1s = spool.tile([C, NPC, H + 2, W], F16)   # zero row top+bottom
            t4s = spool.tile([C, NPC, H, W], F16)
            osb = spool.tile([C, NPC, H, W], F16)

            # zero the t1s padding rows (rows 0 and 57 of each batch elem)
            nc.gpsimd.memset(t1s[:, :, 0, :], 0.0)
            nc.gpsimd.memset(t1s[:, :, H + 1, :], 0.0)

            w1t = wpool.tile([C, 5, C], F16)
            w4t = wpool.tile([C, 3, C], F16)
            w3t = wpool.tile([C, 3], F32)

            # input DMAs: w1 (tiny) and two small x pieces lead so the
            # first chunks unblock as early as possible even when the DMA
            # path has a slow day (a PE gap costs ~2.7x its length: the
            # idle itself plus the HAM clock-gate window it resets).
            # chunk 0 needs x rows <=10, chunk 1 rows <=18.
            nc.sync.dma_start(w1t[:], w1_d[:])
            nc.sync.dma_start(xall[:, 0, 0:10, :], x_d[0, :, 0:10, :])
            nc.sync.dma_start(xall[:, 0, 10:18, :], x_d[0, :, 10:18, :])
            # remainder in two pieces: chunk 2+ would otherwise wait for
            # the whole 1.1MB tail's completion semaphore (~13.5us) and
            # stall the PE mid-stream (seen as 0.2-0.5us hiccups)
            nc.sync.dma_start(xall[:, 0, 18:36, :], x_d[0, :, 18:36, :])
            nc.sync.dma_start(xall[:, 0, 36:H, :], x_d[0, :, 36:H, :])
            nc.sync.dma_start(w4t[:], w4_d[:])
            nc.sync.dma_start(w3t[:], w3_d[:])
            for n in range(1, NPC):
                nc.sync.dma_start(xall[:, n, :, :], x_d[n])

            SUPERS_LAST = [(0, 16), (16, 32), (32, 40), (40, 48), (48, 56)]
            SLICES_LAST = [(0, 15), (15, 31), (31, 39), (39, 47), (47, 56)]

            for n in range(NPC):
                xn = xall[:, n]
                last = n == NPC - 1

                for si, (h0, h1) in enumerate(SUPERS_LAST if last else SUPERS):
                    pt = ptpool.tile([C, 2, 512], F32, name="pt")
                    pb = pbpool.tile([C, 2, 512], F32, name="pb")
                    nq = (h1 - h0) // 8
                    for q in range(nq):
                        c0 = h0 + q * 8
                        # ---- t1: 5-tap conv over H (zero pad via row clip)
                        # center tap e=2 first: always full 8 rows
                        mmsT = []
                        for e in (2, 0, 1, 3, 4):
                            o_lo = max(0, 2 - e - c0)
                            o_hi = min(8, H + 2 - e - c0)
                            mmsT.append(
                                (
                                    w1t[:, e, :],
                                    xn[:, c0 + o_lo + e - 2 : c0 + o_hi + e - 2, :],
                                    pt[:, q, o_lo * W : o_hi * W],
                                )
                            )
                        for i, (lhsT, rhs, outap) in enumerate(mmsT):
                            nc.tensor.matmul(
                                outap,
                                lhsT=lhsT,
                                rhs=rhs,
                                start=(i == 0),
                                stop=(i == len(mmsT) - 1),
                            )
                        # ---- t4: 3 width taps (dilation 2), col-clipped
                        pbq = pb[:, q, 0 : 8 * W].rearrange(
                            "p (h w) -> p h w", h=8, w=W
                        )
                        rows = xn[:, c0 : c0 + 8, :]
                        mmsB = [
                            (w4t[:, 1, :], rows, pbq[:, :, :]),
                            (w4t[:, 0, :], xn[:, c0 : c0 + 8, 0 : W - 2], pbq[:, :, 2:W]),
                            (w4t[:, 2, :], xn[:, c0 : c0 + 8, 2:W], pbq[:, :, 0 : W - 2]),
                        ]
                        for i, (lhsT, rhs, outap) in enumerate(mmsB):
                            nc.tensor.matmul(
                                outap,
                                lhsT=lhsT,
                                rhs=rhs,
                                start=(i == 0),
                                stop=(i == len(mmsB) - 1),
                            )

                    # PSUM -> SBUF fp16 on Act (gpsimd cannot touch PSUM);
                    # one instr per super-chunk spans both banks
                    nc.scalar.copy(
                        t1s[:, n, h0 + 1 : h1 + 1, :],
                        pt[:, 0:nq, 0 : 8 * W],
                    )
                    if not last:
                        nc.scalar.copy(
                            t4s[:, n, h0:h1, :],
                            pb[:, 0:nq, 0 : 8 * W],
                        )

                    # Depthwise+combine chain, all on DVE: three scaled
                    # copies in the 4x tensor_scalar mode, adds and final
                    # multiply in the 2x tensor_tensor mode (the fused
                    # scalar_tensor_tensor is 1x-only: slower).  Row slices
                    # are sized so each becomes ready as its supers land;
                    # the last batch elem uses 14-row quarters so only one
                    # small chain trails the final matmul.
                    def emit_t4s_copy():
                        nc.scalar.copy(
                            t4s[:, n, h0:h1, :],
                            pb[:, 0:nq, 0 : 8 * W],
                        )

                    def emit_dw(r0, r1, tb_on_act=False, mid_copy=None, split_out=False):
                        nr = r1 - r0
                        ta = tpool.tile([C, nr, W], F16, name="ta")
                        tb = tpool.tile([C, nr, W], F16, name="tb")
                        tc_ = tpool.tile([C, nr, W], F16, name="tc_")
                        ts_ = tpool.tile([C, nr, W], F16, name="ts_")
                        t3 = tpool.tile([C, nr, W], F16, name="t3")
                        nc.vector.tensor_scalar_mul(
                            ta[:], t1s[:, n, r0 : r1, :], w3t[:, 0:1]
                        )
                        if tb_on_act:
                            nc.scalar.activation(
                                tb[:],
                                t1s[:, n, r0 + 1 : r1 + 1, :],
                                COPY,
                                scale=w3t[:, 1:2],
                            )
                        else:
                            nc.vector.tensor_scalar_mul(
                                tb[:], t1s[:, n, r0 + 1 : r1 + 1, :], w3t[:, 1:2]
                            )
                        if mid_copy is not None:
                            mid_copy()
                        nc.vector.tensor_scalar_mul(
                            tc_[:], t1s[:, n, r0 + 2 : r1 + 2, :], w3t[:, 2:3]
                        )
                        nc.vector.tensor_tensor(ts_[:], ta[:], tb[:], op=ADD)
                        nc.vector.tensor_tensor(t3[:], ts_[:], tc_[:], op=ADD)
                        cuts = [r0, (r0 + r1) // 2, r1] if split_out else [r0, r1]
                        for a, b in zip(cuts, cuts[1:]):
                            nc.vector.tensor_tensor(
                                osb[:, n, a:b, :],
                                t3[:, a - r0 : b - r0, :],
                                t4s[:, n, a:b, :],
                                op=MUL,
                            )
                            nc.sync.dma_start(
                                out_d[n, :, a:b, :], osb[:, n, a:b, :]
                            )

                    if not last:
                        if si in (1, 3):
                            # elem 2's tail half helps drain the DVE backlog
                            # by pushing its center tap to Act
                            emit_dw(*HALVES[si // 2],
                                    tb_on_act=(n == NPC - 2 and si == 3))
                    else:
                        # last elem: slices aligned to supers so each chain
                        # launches as soon as its own super is copied; the
                        # t4 copy rides the Act queue between tb and the
                        # adds, and only a 9-row chain trails the final
                        # matmul
                        emit_dw(*SLICES_LAST[si], tb_on_act=True,
                                mid_copy=emit_t4s_copy)

    nc.compile()
    return nc


def _get_compiled():
    global _COMPILED
    if _COMPILED is None:
        _COMPILED = _build()
    return _COMPILED


def _prep_weights(w1, w3, w4):
    f16 = ml_dtypes.float16 if hasattr(ml_dtypes, "float16") else np.float16
    w1c = np.asarray(w1, dtype=np.float32)[:, :, :, 0]          # (co, ci, e)
    w1t = np.ascontiguousarray(np.transpose(w1c, (1, 2, 0)))    # (ci, e, co)
    w3c = np.asarray(w3, dtype=np.float32)[:, 0, :, 0]          # (c, d)
    w4c = np.asarray(w4, dtype=np.float32)[:, :, 0, :]          # (ci, k, g)
    w4b = np.ascontiguousarray(np.tile(w4c, (1, 1, C // G)))    # (ci, k, 128)
    return (
        w1t.astype(np.float16),
        w4b.astype(np.float16),
        np.ascontiguousarray(w3c).astype(np.float32),
    )


def kernel(x, w1, w3, w4):
    global LAST_EXEC_NS, LAST_RESULTS
    nc = _get_compiled()
    xh = np.ascontiguousarray(np.asarray(x, dtype=np.float32)).astype(np.float16)
    w1t, w4b, w3v = _prep_weights(w1, w3, w4)

    in_maps = [
        {
            "x_s": np.ascontiguousarray(xh[i * NPC : (i + 1) * NPC]),
            "w1t": w1t,
            "w4b": w4b,
            "w3v": w3v,
        }
        for i in range(NCORES)
    ]
    if TRACE:
        _enable_trace_hook()
    res = bass_utils.run_bass_kernel_spmd(
        nc,
        in_maps,
        core_ids=list(range(NCORES)),
        trace=TRACE,
        tmpdir=TRACE_DIR,
    )
    LAST_EXEC_NS = res.exec_time_ns
    LAST_RESULTS = res
    out = np.concatenate([res.results[i]["out"] for i in range(NCORES)], axis=0)
    return out.astype(np.float32)
